# revision 24
# baseline (speedup 1.0000x reference)
"""Trainium2 Bass kernel for BinaryConv2dBBCU_Down.

Pipeline (per image):
  AvgPool2d(2,2) -> +bias -> sign -> 3x3 binary conv (weights scale*sign(w))
  -> +b0 -> PReLU(alpha) -> +b1

Sharding: pure data parallelism, one image per NeuronCore (batch 8 over 8
cores); conv weights / biases / alphas replicated.

Device math:
  a   = Sign(0.25 * (4-elem pool sum) + move0_bias)       (fp8e4, exactly +-1)
  s   = sum over 9 taps of sign(w)^T @ a_shifted          (exact in fp32 PSUM)
  out = Prelu(scale*s + b0; alpha) + b1                   (fp16 out)
with per-output-channel scale = mean|w| and the Prelu evaluated directly on
the Activation engine (per-partition scale/bias/alpha pointers).

The output is stored as fp16 (halves the write stream; adds < 0.05% relative
error against the 2e-2 gate) and widened to f32 on the host after gather.

Structure: the image is processed in 5 bands of 24 output rows plus one tail
band of 8. Pooling is a single pass over 16 chunks of 8 pooled rows; each
chunk's sign output is written into the owning band's flat padded tile, and
boundary rows are duplicated into the neighbouring band tile so no x row is
ever re-read. The conv uses fp8 DoubleRow matmuls (two taps per instruction)
over the flat padded layout: each output tile is 3 padded rows (N=390
contiguous, the largest whole-row block a 512-f32 PSUM bank holds), four
such blocks live in the four banks of one PSUM tile (12 output rows), and a
tile drains with one Prelu (4D compact read straight from PSUM, fp16 out) +
one tensor_scalar_add (+b1); the garbage border lanes never leave PSUM. A
band's four drains accumulate into one [128, 2, 24, 128] fp16 tile that
ships as a single 1.5MB DMA on the ACT HWDGE queue. The tail band runs the
same way with 2-row N=260 blocks.
"""

import sys

sys.path.insert(0, "/opt/trn_rl_repo")

import numpy as np

B, CIN, COUT, H, W = 8, 128, 256, 256, 256
H2, W2 = H // 2, W // 2  # pooled spatial dims (128, 128)
N_CORES = 8
BAND = 24                 # full band height (output rows)
N_FULL = 5                # full bands; tail band holds the last 8 rows
TAIL = H2 - BAND * N_FULL  # 8
N_CHUNKS = 16
CH = H2 // N_CHUNKS       # 8 pooled rows per chunk

_PROGRAMS: dict = {}


def _build_program(repeats: int = 1):
    import concourse.bacc as bacc
    import concourse.tile as tile
    from concourse import mybir

    import concourse.bass as bass_mod
    f32 = mybir.dt.float32
    fp16 = mybir.dt.float16
    fp8 = mybir.dt.float8e4
    Act = mybir.ActivationFunctionType
    DoubleRow = mybir.MatmulPerfMode.DoubleRow
    WP = W2 + 2          # padded row length (130)

    def flat_len(rows):
        # rows + 2 halo rows, one guard element at each end
        return (rows + 2) * WP + 2

    def band_rows(b):
        return BAND if b < N_FULL else TAIL

    nc = bacc.Bacc("TRN2", target_bir_lowering=False, debug=False,
                   num_devices=N_CORES)
    x_in = nc.declare_dram_parameter("x", [CIN, H, W], f32, isOutput=False)
    wt_in = nc.declare_dram_parameter("wt", [CIN, 9, COUT], fp8,
                                      isOutput=False)
    ct_in = nc.declare_dram_parameter("ct", [128, 9], f32, isOutput=False)
    y_out = nc.declare_dram_parameter("y", [COUT, H2, W2], fp16,
                                      isOutput=True)

    with tile.TileContext(nc) as tc:
        with (
            tc.tile_pool(name="consts", bufs=1) as consts,
            tc.tile_pool(name="xch", bufs=6) as xch_pool,
            tc.tile_pool(name="rs", bufs=3) as rs_pool,
            tc.tile_pool(name="cs", bufs=3) as cs_pool,
            tc.tile_pool(name="apad", bufs=3) as apad_pool,
            tc.tile_pool(name="psum", bufs=2, space="PSUM") as psum_pool,
            tc.tile_pool(name="u", bufs=3) as u_pool,
            tc.tile_pool(name="v", bufs=2) as v_pool,
        ):
            wt_sb = consts.tile([CIN, 9, COUT], fp8)
            nc.sync.dma_start(out=wt_sb[:], in_=wt_in[:])
            ct_sb = consts.tile([128, 9], f32)
            nc.sync.dma_start(out=ct_sb[:], in_=ct_in[:])

            for _rep in range(repeats):
                # Padded sign-activation band tiles: band b local row l holds
                # global pooled row 24b-1+l; col p holds global col p-1.
                apad: dict = {}

                def new_band(b):
                    rows = band_rows(b)
                    fl = flat_len(rows)
                    t = apad_pool.tile([CIN, fl], fp8,
                                       name=f"apad{b}", tag=f"apad{rows}")
                    apad[b] = t
                    vw = t[:, 1:1 + (rows + 2) * WP].rearrange(
                        "p (r c) -> p r c", c=WP)
                    nc.vector.memset(t[:, 0:1], 0.0)
                    nc.vector.memset(t[:, fl - 1:fl], 0.0)
                    nc.vector.memset(vw[:, :, 0:1], 0.0)
                    nc.vector.memset(vw[:, :, W2 + 1:W2 + 2], 0.0)
                    if b == 0:
                        nc.vector.memset(vw[:, 0:1, :], 0.0)
                    if b == N_FULL:
                        nc.vector.memset(vw[:, rows + 1:rows + 2, :], 0.0)
                    return t

                def band_view(b):
                    t = apad[b]
                    rows = band_rows(b)
                    return t[:, 1:1 + (rows + 2) * WP].rearrange(
                        "p (r c) -> p r c", c=WP)

                def sign_to(b, lr, cst, rsl):
                    nc.scalar.activation(
                        out=band_view(b)[:, lr:lr + (rsl.stop - rsl.start),
                                         1:W2 + 1],
                        in_=cst[:, rsl, :], func=Act.Sign,
                        bias=ct_sb[:, 0:1], scale=0.25)

                def emit_chunk(c):
                    # pooled rows 8c .. 8c+7
                    bm = min(c // 3, N_FULL)
                    if bm not in apad:
                        new_band(bm)
                    xt = xch_pool.tile([CIN, 2 * CH, W], f32, name="xt")
                    nc.sync.dma_start(out=xt,
                                      in_=x_in[:, 2 * CH * c:2 * CH * (c + 1), :])
                    # column-pair sum first on DVE, so the GpSimd row-pair
                    # stage reads contiguous 512B row runs instead of
                    # stride-8B element pairs
                    xv = xt.rearrange("p r (w two) -> p r w two", two=2)
                    rt = rs_pool.tile([CIN, 2 * CH, W2], f32, name="rt")
                    nc.vector.tensor_add(out=rt, in0=xv[:, :, :, 0],
                                         in1=xv[:, :, :, 1])
                    rv = rt.rearrange("p (r two) w -> p r two w", two=2)
                    cst = cs_pool.tile([CIN, CH, W2], f32, name="cst")
                    nc.gpsimd.tensor_add(out=cst, in0=rv[:, :, 0, :],
                                         in1=rv[:, :, 1, :])
                    # main write at local rows (global 8c - band start + 1)
                    l = 8 * c - BAND * bm + 1
                    sign_to(bm, l, cst, slice(0, CH))
                    if c % 3 == 0 and bm > 0:
                        # first row is also band bm-1's bottom halo
                        sign_to(bm - 1, band_rows(bm - 1) + 1, cst,
                                slice(0, 1))
                    if c % 3 == 2 and bm + 1 <= N_FULL:
                        # last row is also band bm+1's top halo (row 0)
                        if bm + 1 not in apad:
                            new_band(bm + 1)
                        sign_to(bm + 1, 0, cst, slice(CH - 1, CH))

                vt_by_band: dict = {}

                def get_vt(b):
                    if b not in vt_by_band:
                        rows = band_rows(b)
                        vt_by_band[b] = v_pool.tile(
                            [128, 2, rows, W2], fp16, name="vt",
                            tag=f"vt{rows}")
                    return vt_by_band[b]

                def emit_unit(b, u):
                    # one PSUM tile: 4 row-blocks of RB rows (RB*WP cols),
                    # output rows [RB*4*u, RB*4*(u+1)) of band b, both
                    # channel halves
                    ap_t = apad[b]
                    vt = get_vt(b)
                    RB = 3 if b < N_FULL else 2
                    NB = RB * WP
                    rows_u = 4 * RB
                    for h in (0, 1):
                        sc_ap = ct_sb[:, 1 + 4 * h:2 + 4 * h]
                        b0_ap = ct_sb[:, 2 + 4 * h:3 + 4 * h]
                        al_ap = ct_sb[:, 3 + 4 * h:4 + 4 * h]
                        b1_ap = ct_sb[:, 4 + 4 * h:5 + 4 * h]
                        pt4 = psum_pool.tile([128, 4, 512], f32,
                                             name="pt4", tag="pt4")
                        outs = [pt4[:, k, 0:NB] for k in range(4)]
                        rbase = [rows_u * u + RB * k for k in range(4)]
                        # fp8 DoubleRow: tap pairs (0,1)(2,3)(4,5)(6,7) run
                        # two K=128 contractions per instruction; tap 8 is a
                        # plain fp8 matmul. tap-major keeps lhsT stationary
                        # across the four blocks.
                        for t in (0, 2, 4, 6, 8):
                            ky, kx = divmod(t, 3)
                            dt0 = (ky - 1) * WP + (kx - 1)
                            if t < 8:
                                ky2, kx2 = divmod(t + 1, 3)
                                dpair = (ky2 - ky) * WP + (kx2 - kx)
                                lhs = wt_sb[:, t:t + 2, h * 128:(h + 1) * 128]
                            else:
                                lhs = wt_sb[:, t, h * 128:(h + 1) * 128]
                            for r, po in zip(rbase, outs):
                                base = 1 + (r + 1) * WP + dt0
                                r0 = ap_t[:, base:base + NB]
                                if t < 8:
                                    rhs = bass_mod.AP(
                                        tensor=r0.tensor, offset=r0.offset,
                                        ap=[r0.ap[0], [dpair, 2], r0.ap[1]])
                                    nc.tensor.matmul(po, lhs, rhs,
                                                     start=(t == 0),
                                                     stop=False,
                                                     perf_mode=DoubleRow)
                                else:
                                    nc.tensor.matmul(po, lhs, r0,
                                                     start=False, stop=True)
                        pv = pt4[:, :, 0:NB].rearrange(
                            "p f (r c) -> p f r c", c=WP)[:, :, :, 1:W2 + 1]
                        ut = u_pool.tile([128, 4 * RB * W2], fp16, name="ut",
                                         tag=f"ut{RB}")
                        # out = Prelu(scale*s + b0; alpha), compacted on the
                        # way out of PSUM (4D in-AP, contiguous fp16 out)
                        nc.scalar.activation(out=ut, in_=pv, func=Act.Prelu,
                                             bias=b0_ap, scale=sc_ap,
                                             alpha=al_ap)
                        vo = vt[:, h, rows_u * u:rows_u * (u + 1), :]
                        nc.vector.tensor_scalar_add(
                            out=vo, in0=ut.rearrange("p (r c) -> p r c",
                                                     c=W2),
                            scalar1=b1_ap)

                def ship_band(b):
                    rows = band_rows(b)
                    y0 = BAND * b
                    yv = y_out.rearrange("(h p) r w -> p h r w", h=2)
                    nc.scalar.dma_start(out=yv[:, :, y0:y0 + rows, :],
                                        in_=vt_by_band[b])
                    vt_by_band.pop(b)
                    apad.pop(b)

                # unit (b,0) needs pooled rows to 24b+12 (chunk 3b+1);
                # unit (b,1) needs the bottom halo from chunk 3b+3
                for c in range(N_CHUNKS):
                    emit_chunk(c)
                    if c % 3 == 1:
                        emit_unit(c // 3, 0)
                    elif c % 3 == 0 and c >= 3:
                        emit_unit(c // 3 - 1, 1)
                        ship_band(c // 3 - 1)
                emit_unit(N_FULL, 0)
                ship_band(N_FULL)
    nc.compile()
    return nc


def get_program(repeats: int = 1):
    if repeats not in _PROGRAMS:
        _PROGRAMS[repeats] = _build_program(repeats)
    return _PROGRAMS[repeats]


def host_prep(weight, move0_bias, pr_bias0, prelu_alpha, pr_bias1):
    import ml_dtypes

    w = np.asarray(weight, dtype=np.float32)  # [COUT, CIN, 3, 3]
    sw = np.sign(w).astype(np.float32)
    # lhsT layout [ci, tap, co]
    wt = np.ascontiguousarray(
        np.transpose(sw, (1, 2, 3, 0)).reshape(CIN, 9, COUT)
    ).astype(ml_dtypes.float8_e4m3)

    scale = np.mean(np.abs(w), axis=(1, 2, 3), dtype=np.float32)  # [COUT]
    al = np.asarray(prelu_alpha, dtype=np.float32).reshape(COUT)
    b0 = np.asarray(pr_bias0, dtype=np.float32).reshape(COUT)
    b1 = np.asarray(pr_bias1, dtype=np.float32).reshape(COUT)

    ct = np.zeros((128, 9), dtype=np.float32)
    ct[:, 0] = np.asarray(move0_bias, dtype=np.float32).reshape(CIN)
    for h in (0, 1):
        sl = slice(h * 128, (h + 1) * 128)
        ct[:, 1 + 4 * h] = scale[sl]
        ct[:, 2 + 4 * h] = b0[sl]
        ct[:, 3 + 4 * h] = al[sl]
        ct[:, 4 + 4 * h] = b1[sl]
    return wt, ct


def make_in_maps(x, weight, move0_bias, pr_bias0, prelu_alpha, pr_bias1):
    x = np.asarray(x, dtype=np.float32)
    wt, ct = host_prep(weight, move0_bias, pr_bias0, prelu_alpha, pr_bias1)
    return [{"x": x[c], "wt": wt, "ct": ct} for c in range(N_CORES)]


def kernel(x, weight, move0_bias, pr_bias0, prelu_alpha, pr_bias1):
    from concourse.bass_utils import run_bass_kernel_spmd

    nc = get_program()
    in_maps = make_in_maps(x, weight, move0_bias, pr_bias0, prelu_alpha,
                           pr_bias1)
    res = run_bass_kernel_spmd(nc, in_maps, list(range(N_CORES)))
    y = np.stack([res.results[c]["y"] for c in range(N_CORES)], axis=0)
    return np.ascontiguousarray(y.astype(np.float32))


# revision 25
# speedup vs baseline: 1.0153x; 1.0153x over previous
"""Trainium2 Bass kernel for BinaryConv2dBBCU_Down.

Pipeline (per image):
  AvgPool2d(2,2) -> +bias -> sign -> 3x3 binary conv (weights scale*sign(w))
  -> +b0 -> PReLU(alpha) -> +b1

Sharding: pure data parallelism, one image per NeuronCore (batch 8 over 8
cores); conv weights / biases / alphas replicated.

Device math:
  a   = Sign(0.25 * (4-elem pool sum) + move0_bias)       (fp8e4, exactly +-1)
  s   = sum over 9 taps of sign(w)^T @ a_shifted          (exact in fp32 PSUM)
  out = Prelu(scale*s + b0; alpha) + b1                   (fp16 out)
with per-output-channel scale = mean|w| and the Prelu evaluated directly on
the Activation engine (per-partition scale/bias/alpha pointers).

The output is stored as fp16 (halves the write stream; adds < 0.05% relative
error against the 2e-2 gate) and widened to f32 on the host after gather.

Structure: the image is processed in 5 bands of 24 output rows plus one tail
band of 8. Pooling is a single pass over 16 chunks of 8 pooled rows; each
chunk's sign output is written into the owning band's flat padded tile, and
boundary rows are duplicated into the neighbouring band tile so no x row is
ever re-read. The conv uses fp8 DoubleRow matmuls (two taps per instruction)
over the flat padded layout: each output tile is 3 padded rows (N=390
contiguous, the largest whole-row block a 512-f32 PSUM bank holds), four
such blocks live in the four banks of one PSUM tile (12 output rows), and a
tile drains with one Prelu (4D compact read straight from PSUM, fp16 out) +
one tensor_scalar_add (+b1); the garbage border lanes never leave PSUM. A
band's four drains accumulate into one [128, 2, 24, 128] fp16 tile that
ships as a single 1.5MB DMA on the ACT HWDGE queue. The tail band runs the
same way with 2-row N=260 blocks.
"""

import sys

sys.path.insert(0, "/opt/trn_rl_repo")

import numpy as np

B, CIN, COUT, H, W = 8, 128, 256, 256, 256
H2, W2 = H // 2, W // 2  # pooled spatial dims (128, 128)
N_CORES = 8
BAND = 24                 # full band height (output rows)
N_FULL = 5                # full bands; tail band holds the last 8 rows
TAIL = H2 - BAND * N_FULL  # 8
N_CHUNKS = 16
CH = H2 // N_CHUNKS       # 8 pooled rows per chunk

_PROGRAMS: dict = {}


def _build_program(repeats: int = 1):
    import concourse.bacc as bacc
    import concourse.tile as tile
    from concourse import mybir

    import concourse.bass as bass_mod
    f32 = mybir.dt.float32
    fp16 = mybir.dt.float16
    fp8 = mybir.dt.float8e4
    Act = mybir.ActivationFunctionType
    DoubleRow = mybir.MatmulPerfMode.DoubleRow
    WP = W2 + 2          # padded row length (130)

    def flat_len(rows):
        # rows + 2 halo rows, one guard element at each end
        return (rows + 2) * WP + 2

    def band_rows(b):
        return BAND if b < N_FULL else TAIL

    nc = bacc.Bacc("TRN2", target_bir_lowering=False, debug=False,
                   num_devices=N_CORES)
    x_in = nc.declare_dram_parameter("x", [CIN, H, W], f32, isOutput=False)
    wt_in = nc.declare_dram_parameter("wt", [CIN, 9, COUT], fp8,
                                      isOutput=False)
    ct_in = nc.declare_dram_parameter("ct", [128, 9], f32, isOutput=False)
    y_out = nc.declare_dram_parameter("y", [COUT, H2, W2], fp16,
                                      isOutput=True)

    with tile.TileContext(nc) as tc:
        with (
            tc.tile_pool(name="consts", bufs=1) as consts,
            tc.tile_pool(name="xch", bufs=6) as xch_pool,
            tc.tile_pool(name="rs", bufs=3) as rs_pool,
            tc.tile_pool(name="cs", bufs=3) as cs_pool,
            tc.tile_pool(name="apad", bufs=3) as apad_pool,
            tc.tile_pool(name="psum", bufs=2, space="PSUM") as psum_pool,
            tc.tile_pool(name="u", bufs=3) as u_pool,
            tc.tile_pool(name="v", bufs=2) as v_pool,
        ):
            wt_sb = consts.tile([CIN, 9, COUT], fp8)
            nc.sync.dma_start(out=wt_sb[:], in_=wt_in[:])
            ct_sb = consts.tile([128, 9], f32)
            nc.sync.dma_start(out=ct_sb[:], in_=ct_in[:])

            for _rep in range(repeats):
                # Padded sign-activation band tiles: band b local row l holds
                # global pooled row 24b-1+l; col p holds global col p-1.
                apad: dict = {}

                def new_band(b):
                    rows = band_rows(b)
                    fl = flat_len(rows)
                    t = apad_pool.tile([CIN, fl], fp8,
                                       name=f"apad{b}", tag=f"apad{rows}")
                    apad[b] = t
                    vw = t[:, 1:1 + (rows + 2) * WP].rearrange(
                        "p (r c) -> p r c", c=WP)
                    nc.vector.memset(t[:, 0:1], 0.0)
                    nc.vector.memset(t[:, fl - 1:fl], 0.0)
                    nc.vector.memset(vw[:, :, 0:1], 0.0)
                    nc.vector.memset(vw[:, :, W2 + 1:W2 + 2], 0.0)
                    if b == 0:
                        nc.vector.memset(vw[:, 0:1, :], 0.0)
                    if b == N_FULL:
                        nc.vector.memset(vw[:, rows + 1:rows + 2, :], 0.0)
                    return t

                def band_view(b):
                    t = apad[b]
                    rows = band_rows(b)
                    return t[:, 1:1 + (rows + 2) * WP].rearrange(
                        "p (r c) -> p r c", c=WP)

                def sign_to(b, lr, cst, rsl):
                    nc.scalar.activation(
                        out=band_view(b)[:, lr:lr + (rsl.stop - rsl.start),
                                         1:W2 + 1],
                        in_=cst[:, rsl, :], func=Act.Sign,
                        bias=ct_sb[:, 0:1], scale=0.25)

                def emit_chunk(c):
                    # pooled rows 8c .. 8c+7
                    bm = min(c // 3, N_FULL)
                    if bm not in apad:
                        new_band(bm)
                    xt = xch_pool.tile([CIN, 2 * CH, W], f32, name="xt")
                    nc.sync.dma_start(out=xt,
                                      in_=x_in[:, 2 * CH * c:2 * CH * (c + 1), :])
                    xv = xt.rearrange("p (r two) w -> p r two w", two=2)
                    rt = rs_pool.tile([CIN, CH, W], f32, name="rt")
                    nc.vector.tensor_add(out=rt, in0=xv[:, :, 0, :],
                                         in1=xv[:, :, 1, :])
                    rv = rt.rearrange("p r (w two) -> p r w two", two=2)
                    cst = cs_pool.tile([CIN, CH, W2], f32, name="cst")
                    # column-pair sum on GpSimd; DVE keeps only the row sum
                    nc.gpsimd.tensor_add(out=cst, in0=rv[:, :, :, 0],
                                         in1=rv[:, :, :, 1])
                    # main write at local rows (global 8c - band start + 1)
                    l = 8 * c - BAND * bm + 1
                    sign_to(bm, l, cst, slice(0, CH))
                    if c % 3 == 0 and bm > 0:
                        # first row is also band bm-1's bottom halo
                        sign_to(bm - 1, band_rows(bm - 1) + 1, cst,
                                slice(0, 1))
                    if c % 3 == 2 and bm + 1 <= N_FULL:
                        # last row is also band bm+1's top halo (row 0)
                        if bm + 1 not in apad:
                            new_band(bm + 1)
                        sign_to(bm + 1, 0, cst, slice(CH - 1, CH))

                vt_by_band: dict = {}

                def get_vt(b):
                    if b not in vt_by_band:
                        rows = band_rows(b)
                        vt_by_band[b] = v_pool.tile(
                            [128, 2, rows, W2], fp16, name="vt",
                            tag=f"vt{rows}")
                    return vt_by_band[b]

                def emit_unit(b, u):
                    # one PSUM tile: 4 row-blocks of RB rows (RB*WP cols),
                    # output rows [RB*4*u, RB*4*(u+1)) of band b, both
                    # channel halves
                    ap_t = apad[b]
                    vt = get_vt(b)
                    RB = 3 if b < N_FULL else 2
                    NB = RB * WP
                    rows_u = 4 * RB
                    for h in (0, 1):
                        sc_ap = ct_sb[:, 1 + 4 * h:2 + 4 * h]
                        b0_ap = ct_sb[:, 2 + 4 * h:3 + 4 * h]
                        al_ap = ct_sb[:, 3 + 4 * h:4 + 4 * h]
                        b1_ap = ct_sb[:, 4 + 4 * h:5 + 4 * h]
                        pt4 = psum_pool.tile([128, 4, 512], f32,
                                             name="pt4", tag="pt4")
                        outs = [pt4[:, k, 0:NB] for k in range(4)]
                        rbase = [rows_u * u + RB * k for k in range(4)]
                        # fp8 DoubleRow: tap pairs (0,1)(2,3)(4,5)(6,7) run
                        # two K=128 contractions per instruction; tap 8 is a
                        # plain fp8 matmul. tap-major keeps lhsT stationary
                        # across the four blocks.
                        for t in (0, 2, 4, 6, 8):
                            ky, kx = divmod(t, 3)
                            dt0 = (ky - 1) * WP + (kx - 1)
                            if t < 8:
                                ky2, kx2 = divmod(t + 1, 3)
                                dpair = (ky2 - ky) * WP + (kx2 - kx)
                                lhs = wt_sb[:, t:t + 2, h * 128:(h + 1) * 128]
                            else:
                                lhs = wt_sb[:, t, h * 128:(h + 1) * 128]
                            for r, po in zip(rbase, outs):
                                base = 1 + (r + 1) * WP + dt0
                                r0 = ap_t[:, base:base + NB]
                                if t < 8:
                                    rhs = bass_mod.AP(
                                        tensor=r0.tensor, offset=r0.offset,
                                        ap=[r0.ap[0], [dpair, 2], r0.ap[1]])
                                    nc.tensor.matmul(po, lhs, rhs,
                                                     start=(t == 0),
                                                     stop=False,
                                                     perf_mode=DoubleRow)
                                else:
                                    nc.tensor.matmul(po, lhs, r0,
                                                     start=False, stop=True)
                        pv = pt4[:, :, 0:NB].rearrange(
                            "p f (r c) -> p f r c", c=WP)[:, :, :, 1:W2 + 1]
                        ut = u_pool.tile([128, 4 * RB * W2], fp16, name="ut",
                                         tag=f"ut{RB}")
                        # out = Prelu(scale*s + b0; alpha), compacted on the
                        # way out of PSUM (4D in-AP, contiguous fp16 out)
                        nc.scalar.activation(out=ut, in_=pv, func=Act.Prelu,
                                             bias=b0_ap, scale=sc_ap,
                                             alpha=al_ap)
                        vo = vt[:, h, rows_u * u:rows_u * (u + 1), :]
                        nc.vector.tensor_scalar_add(
                            out=vo, in0=ut.rearrange("p (r c) -> p r c",
                                                     c=W2),
                            scalar1=b1_ap)

                def ship_band(b):
                    rows = band_rows(b)
                    y0 = BAND * b
                    yv = y_out.rearrange("(h p) r w -> p h r w", h=2)
                    nc.scalar.dma_start(out=yv[:, :, y0:y0 + rows, :],
                                        in_=vt_by_band[b])
                    vt_by_band.pop(b)
                    apad.pop(b)

                # unit (b,0) needs pooled rows to 24b+12 (chunk 3b+1);
                # unit (b,1) needs the bottom halo from chunk 3b+3
                for c in range(N_CHUNKS):
                    emit_chunk(c)
                    if c % 3 == 1:
                        emit_unit(c // 3, 0)
                    elif c % 3 == 0 and c >= 3:
                        emit_unit(c // 3 - 1, 1)
                        ship_band(c // 3 - 1)
                emit_unit(N_FULL, 0)
                ship_band(N_FULL)
    nc.compile()
    return nc


def get_program(repeats: int = 1):
    if repeats not in _PROGRAMS:
        _PROGRAMS[repeats] = _build_program(repeats)
    return _PROGRAMS[repeats]


def host_prep(weight, move0_bias, pr_bias0, prelu_alpha, pr_bias1):
    import ml_dtypes

    w = np.asarray(weight, dtype=np.float32)  # [COUT, CIN, 3, 3]
    sw = np.sign(w).astype(np.float32)
    # lhsT layout [ci, tap, co]
    wt = np.ascontiguousarray(
        np.transpose(sw, (1, 2, 3, 0)).reshape(CIN, 9, COUT)
    ).astype(ml_dtypes.float8_e4m3)

    scale = np.mean(np.abs(w), axis=(1, 2, 3), dtype=np.float32)  # [COUT]
    al = np.asarray(prelu_alpha, dtype=np.float32).reshape(COUT)
    b0 = np.asarray(pr_bias0, dtype=np.float32).reshape(COUT)
    b1 = np.asarray(pr_bias1, dtype=np.float32).reshape(COUT)

    ct = np.zeros((128, 9), dtype=np.float32)
    ct[:, 0] = np.asarray(move0_bias, dtype=np.float32).reshape(CIN)
    for h in (0, 1):
        sl = slice(h * 128, (h + 1) * 128)
        ct[:, 1 + 4 * h] = scale[sl]
        ct[:, 2 + 4 * h] = b0[sl]
        ct[:, 3 + 4 * h] = al[sl]
        ct[:, 4 + 4 * h] = b1[sl]
    return wt, ct


def make_in_maps(x, weight, move0_bias, pr_bias0, prelu_alpha, pr_bias1):
    x = np.asarray(x, dtype=np.float32)
    wt, ct = host_prep(weight, move0_bias, pr_bias0, prelu_alpha, pr_bias1)
    return [{"x": x[c], "wt": wt, "ct": ct} for c in range(N_CORES)]


def kernel(x, weight, move0_bias, pr_bias0, prelu_alpha, pr_bias1):
    from concourse.bass_utils import run_bass_kernel_spmd

    nc = get_program()
    in_maps = make_in_maps(x, weight, move0_bias, pr_bias0, prelu_alpha,
                           pr_bias1)
    res = run_bass_kernel_spmd(nc, in_maps, list(range(N_CORES)))
    y = np.stack([res.results[c]["y"] for c in range(N_CORES)], axis=0)
    return np.ascontiguousarray(y.astype(np.float32))
